# revision 30
# baseline (speedup 1.0000x reference)
"""Multi-head attention Bass kernel for Trainium2, 8 NeuronCores.

Problem: B=2, R=16, C=512, E=1024, H=16 heads, D=64.
  q,k,v = x @ w{q,k,v} + b{q,k,v}  (per-head attention)  out = ctx @ wo + bo

Sharding: pure data parallel over the B*R = 32 independent (batch,row)
sequences -> 4 sequences of 512 tokens per core. No collectives.

v3 design (on top of the v2 bf16 pipeline):
  - Q/K projections in fp8e4 with perf_mode=DoubleRow: lhsT [128,2,128]
    and rhs [128,2,512] 3D APs contract TWO 128-feature chunks per
    N=512 span -> 4 spans per 128-feature output chunk instead of 8.
    Host pre-scales wq/wk (and bq/bk) by 16 so the uniform(-1/32,1/32)
    weights land in fp8e4 normal range; the 256x logit scale folds into
    the exp scale. Host-sim predicts 1.56e-2 rel err (gate 2e-2).
  - fp8 weights land p-sliced (one [128, KCH*128] tile per 128-col
    output chunk) so QK(s0,p0) only needs 128KB of weights + x8(s0):
    first matmul ~13us instead of ~19us (startup was DMA-bound).
    Seq 0 therefore runs QK+S first (one QK pair ahead so S covers the
    qt bias-add latency); V groups interleave into pairs 4-7 once
    wv/xt have streamed in; PVs flush via the standard pending queue.
  - PV is col-tiled: both heads of a pair run CONCURRENT M=64 matmuls
    at tile_position (0,0)/(0,64) into one [128,512] psum -> 4 spans
    per pair instead of 8, and ctx pair tiles assemble with a single
    [128,512] DVE multiply (no odd-head staging DMA).
  - softmax denominators (the old ones-column) come from dedicated
    2-way col-tiled M=64 matmuls per pair: ones[128,64] stationary at
    col groups 0-1 / 2-3, each streaming one head's P^T chunk.  All 64
    output partitions of a head's tile receive the SAME l row, i.e.
    the matmul performs the partition-broadcast for free: one [128,512]
    Ln + Exp pair turns the psum directly into the [128,512] 1/l tile
    that the pair's DVE normalize consumes.  No gpsimd broadcasts, no
    row-staging DMAs, and the per-pair chain is ~1.5us (it used to be
    ~6us), which also shrinks the exposed chain at the kernel tail.
  - tail: seq 3's O groups run k=0..5 partials across 4 psum banks
    while the last pair's PV/denominator chain completes, so the PE
    never idles >3.4us (no HAM re-throttle).
"""

import numpy as np
import ml_dtypes

import concourse.bacc as bacc
import concourse.mybir as mybir
import concourse.tile as tile
from concourse import bass_utils

F32 = mybir.dt.float32
BF16 = mybir.dt.bfloat16
FP8 = mybir.dt.float8e4
DR = mybir.MatmulPerfMode.DoubleRow

# The kernel uses both Exp and Ln on ScalarE. Left alone, the table-load
# placement pass picks "exp_and_others" for Exp and "natural_log" for Ln,
# reloading the ACT tables (~2.7us) on every alternation. Restrict both
# functions to the one set that contains them together.
_orig_get_tables = bacc.get_activation_tables


def _combined_exp_ln_tables(arch):
    tabs = _orig_get_tables(arch)
    keep = "natural_log_exp_and_others"
    for name, fns in tabs.items():
        if name != keep:
            fns.discard(mybir.ActivationFunctionType.Exp)
            fns.discard(mybir.ActivationFunctionType.Ln)
    return tabs


bacc.get_activation_tables = _combined_exp_ln_tables

B, R, C, E, H = 2, 16, 512, 1024, 16
D = E // H            # 64
NCORES = 8
SEQS = (B * R) // NCORES   # 4 sequences per core
TCH = C // 128             # 4 token chunks per sequence
KCH = E // 128             # 8 contraction chunks
NCH = E // 512             # 2 output column chunks
PAIRS = H // 2             # 8 head pairs
WS = 16.0                  # host-side fp8 weight scale for wq/wk (and bq/bk)
SCALE = 1.0 / (np.sqrt(D) * WS * WS)   # folded into exp


def build_nc():
    nc = bacc.Bacc("TRN2", debug=False, num_devices=NCORES)

    # bf16 x^T (for the V projection): [seq][128 part][k-chunk * 512 tok]
    xt_d = nc.dram_tensor("xt", [SEQS, 128, KCH * 512], BF16,
                          kind="ExternalInput").ap()
    # fp8 x^T for Q/K: same layout
    x8_d = nc.dram_tensor("x8", [SEQS, 128, KCH * 512], FP8,
                          kind="ExternalInput").ap()
    # fp8 wq/wk, 2 p-slices per tile: [g][128 part][pp*1024 + k*128 + pc]
    w8_d = {}
    for w in ("wq", "wk"):
        w8_d[w] = nc.dram_tensor(w + "8", [PAIRS // 2, 128, 2 * KCH * 128],
                                 FP8, kind="ExternalInput").ap()
    # bf16 wv/wo, 4 k-chunks per tile: [h][128 part][(k%4)*1024 + e]
    w_d = {}
    for w in ("wv", "wo"):
        w_d[w] = nc.dram_tensor(w, [2, 128, 4 * E], BF16,
                                 kind="ExternalInput").ap()
    b_d = {}
    for b in ("bq", "bk", "bv", "bo"):
        b_d[b] = nc.dram_tensor(b, [E], F32, kind="ExternalInput").ap()
    # host-pre-transposed per-partition bias layouts t[p, j] = b[j*128+p]
    bt_d = {}
    for b in ("bq", "bk"):
        bt_d[b] = nc.dram_tensor(b + "t", [128, KCH], F32,
                                 kind="ExternalInput").ap()
    os_d = nc.dram_tensor("os", [SEQS * C, E], F32, kind="ExternalOutput").ap()

    with tile.TileContext(nc) as tc:
        with (
            tc.tile_pool(name="consts", bufs=1) as cpool,
            tc.tile_pool(name="w8pool", bufs=8) as w8p,
            tc.tile_pool(name="wpool", bufs=4) as wpool,
            tc.tile_pool(name="x8pool", bufs=2) as x8p,
            tc.tile_pool(name="xT", bufs=2) as xTp,
            tc.tile_pool(name="vsb", bufs=9) as vp,
            tc.tile_pool(name="qk", bufs=4) as qkp,
            tc.tile_pool(name="pt", bufs=18) as ptp,
            tc.tile_pool(name="nrm", bufs=2) as nrmp,
            tc.tile_pool(name="ctx", bufs=16) as ctxp,
            tc.tile_pool(name="osb", bufs=2) as osbp,
            tc.tile_pool(name="ps_a", bufs=2, space="PSUM") as ps_a,
            tc.tile_pool(name="ps_s", bufs=2, space="PSUM") as ps_s,
            tc.tile_pool(name="ps_c", bufs=2, space="PSUM") as ps_c,
        ):
            # ---------------- constants ----------------
            ones_col = cpool.tile([128, 64], BF16, name="ones_col")
            nc.vector.memset(ones_col[:], 1.0)

            # ------------- input DMAs, in priority order -------------
            # Startup is DMA-bound: QK(s0) runs first and only needs
            # x8(s0) + the fp8 p-tiles, so those go first.  bf16 x and
            # wv arrive while QK/S compute runs; wo only by seq 1.
            def x8_dma(s):
                t = x8p.tile([128, KCH * 512], FP8, name=f"x8_{s}", tag="x8")
                nc.sync.dma_start(t[:], x8_d[s])
                return t

            x8 = x8_dma(0)

            # per-partition bias layouts (host-pre-transposed); issued
            # from the (idle) scalar queue so the startup descriptor
            # stream on the sync queue stays short
            bqt = cpool.tile([128, KCH], F32, name="bqt")
            bkt = cpool.tile([128, KCH], F32, name="bkt")
            nc.sync.dma_start(bqt[:], bt_d["bq"])
            nc.sync.dma_start(bkt[:], bt_d["bk"])

            w8 = {"wq": [], "wk": []}

            def w8_dma(g):
                for wname, eng in (("wq", nc.sync), ("wk", nc.sync)):
                    t = w8p.tile([128, 2 * KCH * 128], FP8,
                                 name=f"{wname}8_{g}", tag="w8")
                    eng.dma_start(t[:], w8_d[wname][g])
                    w8[wname].append(t)

            w8_dma(0)
            w8_dma(1)

            def xT_dma(s, eng=None):
                t = xTp.tile([128, KCH * 512], BF16, name=f"xT{s}", tag="xT")
                (eng or nc.sync).dma_start(t[:], xt_d[s])
                return t

            def load_w(name, eng=None):
                tiles = []
                for h in range(2):
                    t = wpool.tile([128, 4 * E], BF16,
                                   name=f"{name}_{h}", tag=name)
                    (eng or nc.sync).dma_start(t[:], w_d[name][h])
                    tiles.append(t)
                return tiles

            xT = xT_dma(0)
            wv_sb = load_w("wv")
            w8_dma(2)
            w8_dma(3)
            # (xT(0) and wv ride the scalar/gpsimd queues at startup)

            bvr = cpool.tile([1, E], F32, name="bvr")
            bvb = cpool.tile([128, E], F32, name="bvb")
            nc.sync.dma_start(bvr[:], b_d["bv"].rearrange("(o e) -> o e", o=1))
            nc.gpsimd.partition_broadcast(bvb[:], bvr[0:1, :])

            wo_sb = None   # loaded after seq 0 -- needed only from seq 1

            bor = cpool.tile([1, E], F32, name="bor")
            bob = cpool.tile([128, E], F32, name="bob")
            nc.sync.dma_start(bor[:], b_d["bo"].rearrange("(o e) -> o e", o=1))
            nc.gpsimd.partition_broadcast(bob[:], bor[0:1, :])

            # one V-projection psum group (t, n); DVE adds bias -> bf16
            def v_group(s, vt, t, n, xT):
                pv = ps_a.tile([128, 512], F32, name=f"pv{s}_{t}{n}", tag="pj")
                for k in range(KCH):
                    nc.tensor.matmul(
                        pv[:],
                        xT[:, k * 512 + t * 128:k * 512 + (t + 1) * 128],
                        wv_sb[k // 4][:, (k % 4) * E + n * 512:
                                      (k % 4) * E + (n + 1) * 512],
                        start=(k == 0), stop=(k == KCH - 1))
                nc.vector.tensor_tensor(
                    vt[:, n * 512:(n + 1) * 512], pv[:],
                    bvb[:, n * 512:(n + 1) * 512],
                    op=mybir.AluOpType.add)

            def v_tiles(s):
                return [vp.tile([128, E], BF16, name=f"v{s}_{t}", tag="v")
                        for t in range(TCH)]

            def v_proj(s, xT):
                vsb = v_tiles(s)
                for t in range(TCH):
                    for n in range(NCH):
                        v_group(s, vsb[t], t, n, xT)
                return vsb

            # Q^T / K^T for feature pair p via fp8 DoubleRow:
            # psum[128 feat, 512 tok] = sum_kp w8[p] pair kp .T @ x8 pair kp
            def qk_proj(s, p, x8):
                x3 = x8[:].rearrange("a (k n) -> a k n", n=512)
                qkt = {}
                for nm, wsb, bt in (("q", w8["wq"][p // 2], bqt),
                                    ("k", w8["wk"][p // 2], bkt)):
                    w3 = wsb[:, (p % 2) * KCH * 128:
                             (p % 2 + 1) * KCH * 128].rearrange(
                                 "a (k m) -> a k m", m=128)
                    pq = ps_a.tile([128, 512], F32, name=f"pq{nm}{s}_{p}",
                                   tag="pj")
                    for kp in range(KCH // 2):
                        nc.tensor.matmul(
                            pq[:],
                            w3[:, 2 * kp:2 * kp + 2, :],
                            x3[:, 2 * kp:2 * kp + 2, :],
                            start=(kp == 0), stop=(kp == KCH // 2 - 1),
                            perf_mode=DR)
                    qt = qkp.tile([128, 512], BF16, name=f"{nm}T{s}_{p}",
                                  tag="qk")
                    nc.vector.tensor_scalar_add(qt[:], pq[:], bt[:, p:p + 1])
                    qkt[nm] = qt
                return qkt["q"], qkt["k"]

            # S^T chunk-pair cp for head pair p: two row-tiled (concurrent)
            # K=64 matmuls per kj chunk; two kj chunks share a 2-bank psum
            # tile so each ACT exp covers [128,1024].
            def s_alloc(s, p, cp):
                pse = ps_s.tile([128, 1024], F32, name=f"pse{s}{p}{cp}", tag="s")
                pso = ps_s.tile([128, 1024], F32, name=f"pso{s}{p}{cp}", tag="s")
                return pse, pso

            def s_span(pse, pso, cp, ci, QT, KT):
                c = 2 * cp + ci
                nc.tensor.matmul(
                    pse[:, ci * 512:(ci + 1) * 512],
                    KT[0:64, c * 128:(c + 1) * 128], QT[0:64, :],
                    start=True, stop=True, tile_position=(0, 0))
                nc.tensor.matmul(
                    pso[:, ci * 512:(ci + 1) * 512],
                    KT[64:128, c * 128:(c + 1) * 128], QT[64:128, :],
                    start=True, stop=True, tile_position=(64, 0))

            def s_block(s, p, cp, QT, KT):
                pse, pso = s_alloc(s, p, cp)
                s_span(pse, pso, cp, 0, QT, KT)
                s_span(pse, pso, cp, 1, QT, KT)
                return pse, pso

            def s_exps(s, p, cp, pse, pso):
                out = []
                for hh, ps_t in ((0, pse), (1, pso)):
                    pt_t = ptp.tile([128, 1024], BF16,
                                    name=f"pt{s}{p}{cp}{hh}", tag="pt")
                    nc.scalar.activation(
                        pt_t[:], ps_t[:],
                        mybir.ActivationFunctionType.Exp, scale=float(SCALE))
                    out.append(pt_t)
                return out

            # col-tiled PV + softmax denominator for pair p.  Both heads
            # run concurrent M=64 matmuls (col groups 0-1 / 2-3); the
            # ones[128,64] stationary makes every output partition of a
            # head's denominator tile carry the same l row, so the
            # matmul IS the partition-broadcast.  One Ln+Exp [128,512]
            # pair turns it into 1/l and a single DVE multiply
            # normalizes the whole [128,512] ctx pair tile.
            def pv_pair_mm(s, p, PT2, vsb, pl_pool=None, pl_tag="pj"):
                pc = ps_c.tile([128, 512], F32, name=f"pc{s}{p}", tag="c")
                for c in range(TCH):
                    for hh in range(2):
                        h = 2 * p + hh
                        nc.tensor.matmul(
                            pc[64 * hh:64 * hh + 64, :],
                            vsb[c][:, h * D:(h + 1) * D],
                            PT2[hh][c // 2][:, (c % 2) * 512:(c % 2 + 1) * 512],
                            start=(c == 0), stop=(c == TCH - 1),
                            tile_position=(0, 64 * hh),
                            skip_group_check=True)
                pl = (pl_pool or ps_a).tile([128, 512], F32,
                                            name=f"pl{s}{p}", tag=pl_tag)
                for c in range(TCH):
                    for hh in range(2):
                        nc.tensor.matmul(
                            pl[64 * hh:64 * hh + 64, :],
                            ones_col[:],
                            PT2[hh][c // 2][:, (c % 2) * 512:(c % 2 + 1) * 512],
                            start=(c == 0), stop=(c == TCH - 1),
                            tile_position=(0, 64 * hh),
                            skip_group_check=True)
                return pc, pl

            def pv_pair_chain(s, p, pc, pl, out):
                nt = nrmp.tile([128, 1024], F32, name=f"nt{s}{p}", tag="nt")
                nc.scalar.activation(nt[:, 0:512], pl[:],
                                     mybir.ActivationFunctionType.Ln)
                nc.scalar.activation(nt[:, 512:1024], nt[:, 0:512],
                                     mybir.ActivationFunctionType.Exp,
                                     scale=-1.0)
                ctile = ctxp.tile([128, 512], BF16, name=f"ctx{s}_{p}", tag="ctx")
                nc.vector.tensor_tensor(
                    ctile[:], pc[:], nt[:, 512:1024], op=mybir.AluOpType.mult)
                out.append(ctile)

            # output projection group g=(t,n) of seq s
            def o_finish(s, g, po):
                t, n = g // NCH, g % NCH
                ob = osbp.tile([128, 512], F32, name=f"ob{s}{t}{n}", tag="ob")
                nc.vector.tensor_tensor(
                    ob[:], po[:, 0:512], bob[:, n * 512:(n + 1) * 512],
                    op=mybir.AluOpType.add)
                nc.sync.dma_start(
                    os_d[s * C + t * 128: s * C + (t + 1) * 128,
                         n * 512:(n + 1) * 512],
                    ob[:])

            def o_chunks(g, po, ctx_tiles, ks, stop):
                t, n = g // NCH, g % NCH
                for k in ks:
                    nc.tensor.matmul(
                        po[:],
                        ctx_tiles[k][:, t * 128:(t + 1) * 128],
                        wo_sb[k // 4][:, (k % 4) * E + n * 512:
                                      (k % 4) * E + (n + 1) * 512],
                        start=(k == 0), stop=(stop and k == ks[-1]),
                        skip_group_check=True)

            def o_group(s, g, ctx_tiles):
                po = ps_a.tile([128, 512], F32, name=f"po{s}{g}", tag="pj")
                o_chunks(g, po, ctx_tiles, list(range(KCH)), True)
                o_finish(s, g, po)

            # ---------------- main pipeline ----------------
            # Each pair's PV/denominator unit is appended to `pending`
            # and emitted inside a LATER pair's slot so the in-order PE
            # stream never waits on the exps it consumes.
            prev_ctx = None            # seq s-1 ctx tiles for O-proj
            pending = []               # [(s, p, PT2, vsb, out), ...]

            def flush_one_pending(pl_pool=None, pl_tag="pj"):
                if pending:
                    s_, p_, PT2_, vsb_, out_ = pending.pop(0)
                    pc, pl = pv_pair_mm(s_, p_, PT2_, vsb_, pl_pool, pl_tag)
                    pv_pair_chain(s_, p_, pc, pl, out_)

            def make_flush_spans():
                # span-level closures for the pending PV/denominator unit
                # so its matmuls can interleave between the S spans (the
                # S row-tile LDWEIGHTS pairs can't both hide behind their
                # own 213ns span; a partner span gives them the window).
                if not pending:
                    return [], None
                s_, p_, PT2_, vsb_, out_ = pending.pop(0)
                pc = ps_c.tile([128, 512], F32, name=f"pc{s_}{p_}", tag="c")
                box = {}

                def pv_span(c):
                    for hh in range(2):
                        h = 2 * p_ + hh
                        nc.tensor.matmul(
                            pc[64 * hh:64 * hh + 64, :],
                            vsb_[c][:, h * D:(h + 1) * D],
                            PT2_[hh][c // 2][:,
                                             (c % 2) * 512:(c % 2 + 1) * 512],
                            start=(c == 0), stop=(c == TCH - 1),
                            tile_position=(0, 64 * hh),
                            skip_group_check=True)

                def pl_span(c):
                    if "pl" not in box:
                        box["pl"] = ps_a.tile([128, 512], F32,
                                              name=f"pl{s_}{p_}", tag="pj")
                    for hh in range(2):
                        nc.tensor.matmul(
                            box["pl"][64 * hh:64 * hh + 64, :],
                            ones_col[:],
                            PT2_[hh][c // 2][:,
                                             (c % 2) * 512:(c % 2 + 1) * 512],
                            start=(c == 0), stop=(c == TCH - 1),
                            tile_position=(0, 64 * hh),
                            skip_group_check=True)

                spans = ([lambda c=c: pv_span(c) for c in range(TCH)]
                         + [lambda c=c: pl_span(c) for c in range(TCH)])
                chain = lambda: pv_pair_chain(s_, p_, pc, box["pl"], out_)
                return spans, chain

            def flush_chains():
                # the Ln/Exp/normalize chain is emitted at the END of the
                # slot so it sits BEHIND the slot's exps in the strict-
                # FIFO ACT queue (the exps gate ps_s recycling; the chain
                # has a whole extra slot of slack).
                while chains:
                    pv_pair_chain(*chains.pop(0))

            for s in range(SEQS):
                ctx_tiles = []
                if s == 0:
                    # seq 0: QK/S first (fp8 weights arrive first), one
                    # QK pair ahead so S(p) covers qt bias-add latency.
                    # All 8 V groups are emitted at p==4 (wv/xt have
                    # streamed in by then); PV flushes start at p==5
                    # (flush-first in the slot: those pendings' chains
                    # are long done and the PT pool needs the slots).
                    vsb = v_tiles(0)
                    qkt = {0: qk_proj(0, 0, x8)}
                    for p in range(PAIRS):
                        if p + 1 < PAIRS:
                            qkt[p + 1] = qk_proj(0, p + 1, x8)
                        if p == 4:
                            for t in range(TCH):
                                v_group(0, vsb[t], t, 0, xT)
                                v_group(0, vsb[t], t, 1, xT)
                        QT, KT = qkt.pop(p)
                        if p >= 4:
                            flush_one_pending()
                        pse0, pso0 = s_block(0, p, 0, QT, KT)
                        if p >= 4:
                            flush_one_pending()
                        pts0 = s_exps(0, p, 0, pse0, pso0)
                        pse1, pso1 = s_block(0, p, 1, QT, KT)
                        pts1 = s_exps(0, p, 1, pse1, pso1)
                        pending.append(
                            (0, p, [[pts0[0], pts1[0]], [pts0[1], pts1[1]]],
                             vsb, ctx_tiles))
                    # drain the remaining pendings
                    flush_one_pending()
                    x8 = x8_dma(1)
                    flush_one_pending()
                    xT = xT_dma(1)
                    wo_sb = load_w("wo")
                    flush_one_pending()
                else:
                    vsb = v_proj(s, xT)
                    # the cross-seq pending flushes here: V(s)'s matmuls
                    # covered the last exps of seq s-1, so its PV never
                    # stalls the PE.
                    flush_one_pending()
                    if s + 1 < SEQS:
                        x8_next = x8_dma(s + 1)
                        xT_next = xT_dma(s + 1)
                    for p in range(PAIRS):
                        QT, KT = qk_proj(s, p, x8)
                        spans, chain = make_flush_spans()
                        # O groups of seq s-1 between the K projection and
                        # S: covers the KT bias-add DVE latency.
                        if p == 1:
                            o_group(s - 1, 0, prev_ctx)
                            o_group(s - 1, 1, prev_ctx)
                        elif p >= 2:
                            o_group(s - 1, p, prev_ctx)
                        pse0, pso0 = s_alloc(s, p, 0)
                        if spans:
                            spans[0]()
                            s_span(pse0, pso0, 0, 0, QT, KT)
                            spans[1]()
                            s_span(pse0, pso0, 0, 1, QT, KT)
                            for sp in spans[2:]:
                                sp()
                            chain()
                        else:
                            s_span(pse0, pso0, 0, 0, QT, KT)
                            s_span(pse0, pso0, 0, 1, QT, KT)
                        pts0 = s_exps(s, p, 0, pse0, pso0)
                        pse1, pso1 = s_block(s, p, 1, QT, KT)
                        pts1 = s_exps(s, p, 1, pse1, pso1)
                        pending.append(
                            (s, p, [[pts0[0], pts1[0]], [pts0[1], pts1[1]]],
                             vsb, ctx_tiles))
                    if s + 1 < SEQS:
                        x8 = x8_next
                        xT = xT_next
                prev_ctx = ctx_tiles

            # Tail flush: pair 7's PV/denominator unit is still
            # pending and seq 3's O projection remains.  Two O groups run
            # k=0..5 partials (they only need ctx pairs 0..5) around the
            # pending unit to cover its exps/chain; the tail pl comes
            # from ps_s (free once the last exps drain -- ps_a slots are
            # held by the partials until their o_finish).
            s3 = SEQS - 1

            def o_partial(g, pool, tag):
                po = pool.tile([128, 512], F32, name=f"pot{g}", tag=tag)
                o_chunks(g, po, prev_ctx, list(range(KCH - 2)), False)
                return po

            po0 = o_partial(0, ps_a, "pj")
            flush_one_pending(ps_s, "s")
            po1 = o_partial(1, ps_a, "pj")
            tail_pos = ((0, po0), (1, po1))
            for g, po in tail_pos:
                o_chunks(g, po, prev_ctx, [KCH - 2], False)
            for g, po in tail_pos:
                o_chunks(g, po, prev_ctx, [KCH - 1], True)
                o_finish(s3, g, po)
            for g in range(2, TCH * NCH):
                o_group(s3, g, prev_ctx)

    nc.compile()
    return nc


_NC_CACHE = {}


def get_nc():
    if "nc" not in _NC_CACHE:
        _NC_CACHE["nc"] = build_nc()
    return _NC_CACHE["nc"]


def make_in_maps(x, wq, bq, wk, bk, wv, bv, wo, bo):
    bf = ml_dtypes.bfloat16
    f8 = ml_dtypes.float8_e4m3
    x = np.asarray(x, dtype=np.float32)
    args = {}
    # bf16 wv/wo, 4 k-chunks per tile:
    # w[h, kf, (k%4)*1024 + e] = w[k*128+kf, e],  k = 4h..4h+3
    for n, v in (("wv", wv), ("wo", wo)):
        wb = np.asarray(v, dtype=np.float32).astype(bf)
        args[n] = np.ascontiguousarray(
            wb.reshape(2, 4, 128, E).transpose(0, 2, 1, 3)
            .reshape(2, 128, 4 * E))
    for n, v in (("bv", bv), ("bo", bo)):
        args[n] = np.asarray(v, dtype=np.float32)
    # fp8 Q/K weights, scaled by WS, 2 p-slices per tile:
    # w8[g, kf, pp*1024 + k*128 + pc] = (w*WS)[k*128+kf, (2g+pp)*128+pc]
    for n, v in (("wq8", wq), ("wk8", wk)):
        w8 = (np.asarray(v, np.float32) * WS).astype(f8)
        args[n] = np.ascontiguousarray(
            w8.reshape(KCH, 128, PAIRS // 2, 2, 128)
            .transpose(2, 1, 3, 0, 4).reshape(PAIRS // 2, 128, 2 * KCH * 128))
    for n, v in (("bq", bq), ("bk", bk)):
        b16 = np.asarray(v, dtype=np.float32) * WS
        args[n] = b16
        args[n + "t"] = np.ascontiguousarray(b16.reshape(KCH, 128).T)
    xf = x.reshape(B * R, C, E)
    in_maps = []
    for c in range(NCORES):
        m = dict(args)
        xc = xf[c * SEQS:(c + 1) * SEQS]                  # [4, 512, 1024]
        # x^T per seq: t8[s, kf, k*512+t] = x[s, t, k*128+kf]
        xtr = (xc.transpose(0, 2, 1).reshape(SEQS, KCH, 128, C)
               .transpose(0, 2, 1, 3).reshape(SEQS, 128, KCH * C))
        m["xt"] = np.ascontiguousarray(xtr.astype(bf))
        m["x8"] = np.ascontiguousarray(xtr.astype(f8))
        in_maps.append(m)
    return in_maps


def kernel(x, wq, bq, wk, bk, wv, bv, wo, bo):
    in_maps = make_in_maps(x, wq, bq, wk, bk, wv, bv, wo, bo)
    nc = get_nc()
    res = bass_utils.run_bass_kernel_spmd(
        nc, in_maps, core_ids=list(range(NCORES)))
    out = np.concatenate(
        [res.results[c]["os"].reshape(SEQS, C, E) for c in range(NCORES)], axis=0)
    return out.reshape(B, R, C, E).astype(np.float32)


# revision 31
# speedup vs baseline: 1.0320x; 1.0320x over previous
"""Multi-head attention Bass kernel for Trainium2, 8 NeuronCores.

Problem: B=2, R=16, C=512, E=1024, H=16 heads, D=64.
  q,k,v = x @ w{q,k,v} + b{q,k,v}  (per-head attention)  out = ctx @ wo + bo

Sharding: pure data parallel over the B*R = 32 independent (batch,row)
sequences -> 4 sequences of 512 tokens per core. No collectives.

v3 design (on top of the v2 bf16 pipeline):
  - Q/K projections in fp8e4 with perf_mode=DoubleRow: lhsT [128,2,128]
    and rhs [128,2,512] 3D APs contract TWO 128-feature chunks per
    N=512 span -> 4 spans per 128-feature output chunk instead of 8.
    Host pre-scales wq/wk (and bq/bk) by 16 so the uniform(-1/32,1/32)
    weights land in fp8e4 normal range; the 256x logit scale folds into
    the exp scale. Host-sim predicts 1.56e-2 rel err (gate 2e-2).
  - fp8 weights land p-sliced (one [128, KCH*128] tile per 128-col
    output chunk) so QK(s0,p0) only needs 128KB of weights + x8(s0):
    first matmul ~13us instead of ~19us (startup was DMA-bound).
    Seq 0 therefore runs QK+S first (one QK pair ahead so S covers the
    qt bias-add latency); V groups interleave into pairs 4-7 once
    wv/xt have streamed in; PVs flush via the standard pending queue.
  - PV is col-tiled: both heads of a pair run CONCURRENT M=64 matmuls
    at tile_position (0,0)/(0,64) into one [128,512] psum -> 4 spans
    per pair instead of 8, and ctx pair tiles assemble with a single
    [128,512] DVE multiply (no odd-head staging DMA).
  - softmax denominators (the old ones-column) come from dedicated
    2-way col-tiled M=64 matmuls per pair: ones[128,64] stationary at
    col groups 0-1 / 2-3, each streaming one head's P^T chunk.  All 64
    output partitions of a head's tile receive the SAME l row, i.e.
    the matmul performs the partition-broadcast for free: one [128,512]
    Ln + Exp pair turns the psum directly into the [128,512] 1/l tile
    that the pair's DVE normalize consumes.  No gpsimd broadcasts, no
    row-staging DMAs, and the per-pair chain is ~1.5us (it used to be
    ~6us), which also shrinks the exposed chain at the kernel tail.
  - tail: seq 3's O groups run k=0..5 partials across 4 psum banks
    while the last pair's PV/denominator chain completes, so the PE
    never idles >3.4us (no HAM re-throttle).
"""

import numpy as np
import ml_dtypes

import concourse.bacc as bacc
import concourse.mybir as mybir
import concourse.tile as tile
from concourse import bass_utils

F32 = mybir.dt.float32
BF16 = mybir.dt.bfloat16
FP8 = mybir.dt.float8e4
DR = mybir.MatmulPerfMode.DoubleRow

# The kernel uses both Exp and Ln on ScalarE. Left alone, the table-load
# placement pass picks "exp_and_others" for Exp and "natural_log" for Ln,
# reloading the ACT tables (~2.7us) on every alternation. Restrict both
# functions to the one set that contains them together.
_orig_get_tables = bacc.get_activation_tables


def _combined_exp_ln_tables(arch):
    tabs = _orig_get_tables(arch)
    keep = "natural_log_exp_and_others"
    for name, fns in tabs.items():
        if name != keep:
            fns.discard(mybir.ActivationFunctionType.Exp)
            fns.discard(mybir.ActivationFunctionType.Ln)
    return tabs


bacc.get_activation_tables = _combined_exp_ln_tables

B, R, C, E, H = 2, 16, 512, 1024, 16
D = E // H            # 64
NCORES = 8
SEQS = (B * R) // NCORES   # 4 sequences per core
TCH = C // 128             # 4 token chunks per sequence
KCH = E // 128             # 8 contraction chunks
NCH = E // 512             # 2 output column chunks
PAIRS = H // 2             # 8 head pairs
WS = 16.0                  # host-side fp8 weight scale for wq/wk (and bq/bk)
SCALE = 1.0 / (np.sqrt(D) * WS * WS)   # folded into exp


def build_nc():
    nc = bacc.Bacc("TRN2", debug=False, num_devices=NCORES)

    # bf16 x^T (for the V projection): [seq][128 part][k-chunk * 512 tok]
    xt_d = nc.dram_tensor("xt", [SEQS, 128, KCH * 512], BF16,
                          kind="ExternalInput").ap()
    # fp8 x^T for Q/K: same layout
    x8_d = nc.dram_tensor("x8", [SEQS, 128, KCH * 512], FP8,
                          kind="ExternalInput").ap()
    # fp8 wq/wk, 2 p-slices per tile: [g][128 part][pp*1024 + k*128 + pc]
    w8_d = {}
    for w in ("wq", "wk"):
        w8_d[w] = nc.dram_tensor(w + "8", [PAIRS // 2, 128, 2 * KCH * 128],
                                 FP8, kind="ExternalInput").ap()
    # bf16 wv/wo, 4 k-chunks per tile: [h][128 part][(k%4)*1024 + e]
    w_d = {}
    for w in ("wv", "wo"):
        w_d[w] = nc.dram_tensor(w, [2, 128, 4 * E], BF16,
                                 kind="ExternalInput").ap()
    b_d = {}
    for b in ("bq", "bk", "bv", "bo"):
        b_d[b] = nc.dram_tensor(b, [E], F32, kind="ExternalInput").ap()
    # host-pre-transposed per-partition bias layouts t[p, j] = b[j*128+p]
    bt_d = {}
    for b in ("bq", "bk"):
        bt_d[b] = nc.dram_tensor(b + "t", [128, KCH], F32,
                                 kind="ExternalInput").ap()
    os_d = nc.dram_tensor("os", [SEQS * C, E], F32, kind="ExternalOutput").ap()

    with tile.TileContext(nc) as tc:
        with (
            tc.tile_pool(name="consts", bufs=1) as cpool,
            tc.tile_pool(name="w8pool", bufs=8) as w8p,
            tc.tile_pool(name="wpool", bufs=4) as wpool,
            tc.tile_pool(name="x8pool", bufs=2) as x8p,
            tc.tile_pool(name="xT", bufs=2) as xTp,
            tc.tile_pool(name="vsb", bufs=9) as vp,
            tc.tile_pool(name="qk", bufs=4) as qkp,
            tc.tile_pool(name="pt", bufs=18) as ptp,
            tc.tile_pool(name="nrm", bufs=2) as nrmp,
            tc.tile_pool(name="ctx", bufs=16) as ctxp,
            tc.tile_pool(name="osb", bufs=2) as osbp,
            tc.tile_pool(name="ps_a", bufs=2, space="PSUM") as ps_a,
            tc.tile_pool(name="ps_s", bufs=2, space="PSUM") as ps_s,
            tc.tile_pool(name="ps_c", bufs=2, space="PSUM") as ps_c,
        ):
            # ---------------- constants ----------------
            ones_col = cpool.tile([128, 64], BF16, name="ones_col")
            nc.vector.memset(ones_col[:], 1.0)

            # ------------- input DMAs, in priority order -------------
            # Startup is DMA-bound: QK(s0) runs first and only needs
            # x8(s0) + the fp8 p-tiles, so those go first.  bf16 x and
            # wv arrive while QK/S compute runs; wo only by seq 1.
            def x8_dma(s):
                t = x8p.tile([128, KCH * 512], FP8, name=f"x8_{s}", tag="x8")
                nc.sync.dma_start(t[:], x8_d[s])
                return t

            x8 = x8_dma(0)

            # per-partition bias layouts (host-pre-transposed); issued
            # from the (idle) scalar queue so the startup descriptor
            # stream on the sync queue stays short
            bqt = cpool.tile([128, KCH], F32, name="bqt")
            bkt = cpool.tile([128, KCH], F32, name="bkt")
            nc.sync.dma_start(bqt[:], bt_d["bq"])
            nc.sync.dma_start(bkt[:], bt_d["bk"])

            w8 = {"wq": [], "wk": []}

            def w8_dma(g):
                for wname, eng in (("wq", nc.sync), ("wk", nc.sync)):
                    t = w8p.tile([128, 2 * KCH * 128], FP8,
                                 name=f"{wname}8_{g}", tag="w8")
                    eng.dma_start(t[:], w8_d[wname][g])
                    w8[wname].append(t)

            w8_dma(0)
            w8_dma(1)

            def xT_dma(s, eng=None):
                t = xTp.tile([128, KCH * 512], BF16, name=f"xT{s}", tag="xT")
                (eng or nc.sync).dma_start(t[:], xt_d[s])
                return t

            def load_w(name, eng=None):
                tiles = []
                for h in range(2):
                    t = wpool.tile([128, 4 * E], BF16,
                                   name=f"{name}_{h}", tag=name)
                    (eng or nc.sync).dma_start(t[:], w_d[name][h])
                    tiles.append(t)
                return tiles

            xT = xT_dma(0)
            wv_sb = load_w("wv")
            w8_dma(2)
            w8_dma(3)
            # (xT(0) and wv ride the scalar/gpsimd queues at startup)

            bvr = cpool.tile([1, E], F32, name="bvr")
            bvb = cpool.tile([128, E], F32, name="bvb")
            nc.sync.dma_start(bvr[:], b_d["bv"].rearrange("(o e) -> o e", o=1))
            nc.gpsimd.partition_broadcast(bvb[:], bvr[0:1, :])

            wo_sb = None   # loaded after seq 0 -- needed only from seq 1

            bor = cpool.tile([1, E], F32, name="bor")
            bob = cpool.tile([128, E], F32, name="bob")
            nc.sync.dma_start(bor[:], b_d["bo"].rearrange("(o e) -> o e", o=1))
            nc.gpsimd.partition_broadcast(bob[:], bor[0:1, :])

            # one V-projection psum group (t, n); DVE adds bias -> bf16
            def v_group(s, vt, t, n, xT):
                pv = ps_a.tile([128, 512], F32, name=f"pv{s}_{t}{n}", tag="pj")
                for k in range(KCH):
                    nc.tensor.matmul(
                        pv[:],
                        xT[:, k * 512 + t * 128:k * 512 + (t + 1) * 128],
                        wv_sb[k // 4][:, (k % 4) * E + n * 512:
                                      (k % 4) * E + (n + 1) * 512],
                        start=(k == 0), stop=(k == KCH - 1))
                nc.vector.tensor_tensor(
                    vt[:, n * 512:(n + 1) * 512], pv[:],
                    bvb[:, n * 512:(n + 1) * 512],
                    op=mybir.AluOpType.add)

            def v_tiles(s):
                return [vp.tile([128, E], BF16, name=f"v{s}_{t}", tag="v")
                        for t in range(TCH)]

            def v_proj(s, xT):
                vsb = v_tiles(s)
                for t in range(TCH):
                    for n in range(NCH):
                        v_group(s, vsb[t], t, n, xT)
                return vsb

            # Q^T / K^T for feature pair p via fp8 DoubleRow:
            # psum[128 feat, 512 tok] = sum_kp w8[p] pair kp .T @ x8 pair kp
            def qk_proj(s, p, x8):
                x3 = x8[:].rearrange("a (k n) -> a k n", n=512)
                qkt = {}
                for nm, wsb, bt in (("q", w8["wq"][p // 2], bqt),
                                    ("k", w8["wk"][p // 2], bkt)):
                    w3 = wsb[:, (p % 2) * KCH * 128:
                             (p % 2 + 1) * KCH * 128].rearrange(
                                 "a (k m) -> a k m", m=128)
                    pq = ps_a.tile([128, 512], F32, name=f"pq{nm}{s}_{p}",
                                   tag="pj")
                    for kp in range(KCH // 2):
                        nc.tensor.matmul(
                            pq[:],
                            w3[:, 2 * kp:2 * kp + 2, :],
                            x3[:, 2 * kp:2 * kp + 2, :],
                            start=(kp == 0), stop=(kp == KCH // 2 - 1),
                            perf_mode=DR)
                    qt = qkp.tile([128, 512], BF16, name=f"{nm}T{s}_{p}",
                                  tag="qk")
                    nc.vector.tensor_scalar_add(qt[:], pq[:], bt[:, p:p + 1])
                    qkt[nm] = qt
                return qkt["q"], qkt["k"]

            # S^T chunk-pair cp for head pair p: two row-tiled (concurrent)
            # K=64 matmuls per kj chunk; two kj chunks share a 2-bank psum
            # tile so each ACT exp covers [128,1024].
            def s_alloc(s, p, cp):
                pse = ps_s.tile([128, 1024], F32, name=f"pse{s}{p}{cp}", tag="s")
                pso = ps_s.tile([128, 1024], F32, name=f"pso{s}{p}{cp}", tag="s")
                return pse, pso

            def s_span(pse, pso, cp, ci, QT, KT):
                c = 2 * cp + ci
                nc.tensor.matmul(
                    pse[:, ci * 512:(ci + 1) * 512],
                    KT[0:64, c * 128:(c + 1) * 128], QT[0:64, :],
                    start=True, stop=True, tile_position=(0, 0))
                nc.tensor.matmul(
                    pso[:, ci * 512:(ci + 1) * 512],
                    KT[64:128, c * 128:(c + 1) * 128], QT[64:128, :],
                    start=True, stop=True, tile_position=(64, 0))

            def s_block(s, p, cp, QT, KT):
                pse, pso = s_alloc(s, p, cp)
                s_span(pse, pso, cp, 0, QT, KT)
                s_span(pse, pso, cp, 1, QT, KT)
                return pse, pso

            def s_exps(s, p, cp, pse, pso):
                out = []
                for hh, ps_t in ((0, pse), (1, pso)):
                    pt_t = ptp.tile([128, 1024], BF16,
                                    name=f"pt{s}{p}{cp}{hh}", tag="pt")
                    nc.scalar.activation(
                        pt_t[:], ps_t[:],
                        mybir.ActivationFunctionType.Exp, scale=float(SCALE))
                    out.append(pt_t)
                return out

            # col-tiled PV + softmax denominator for pair p.  Both heads
            # run concurrent M=64 matmuls (col groups 0-1 / 2-3); the
            # ones[128,64] stationary makes every output partition of a
            # head's denominator tile carry the same l row, so the
            # matmul IS the partition-broadcast.  One Ln+Exp [128,512]
            # pair turns it into 1/l and a single DVE multiply
            # normalizes the whole [128,512] ctx pair tile.
            def pv_pair_mm(s, p, PT2, vsb, pl_pool=None, pl_tag="pj"):
                pc = ps_c.tile([128, 512], F32, name=f"pc{s}{p}", tag="c")
                for c in range(TCH):
                    for hh in range(2):
                        h = 2 * p + hh
                        nc.tensor.matmul(
                            pc[64 * hh:64 * hh + 64, :],
                            vsb[c][:, h * D:(h + 1) * D],
                            PT2[hh][c // 2][:, (c % 2) * 512:(c % 2 + 1) * 512],
                            start=(c == 0), stop=(c == TCH - 1),
                            tile_position=(0, 64 * hh),
                            skip_group_check=True)
                pl = (pl_pool or ps_a).tile([128, 512], F32,
                                            name=f"pl{s}{p}", tag=pl_tag)
                for c in range(TCH):
                    for hh in range(2):
                        nc.tensor.matmul(
                            pl[64 * hh:64 * hh + 64, :],
                            ones_col[:],
                            PT2[hh][c // 2][:, (c % 2) * 512:(c % 2 + 1) * 512],
                            start=(c == 0), stop=(c == TCH - 1),
                            tile_position=(0, 64 * hh),
                            skip_group_check=True)
                return pc, pl

            def pv_pair_chain(s, p, pc, pl, out):
                nt = nrmp.tile([128, 1024], F32, name=f"nt{s}{p}", tag="nt")
                nc.scalar.activation(nt[:, 0:512], pl[:],
                                     mybir.ActivationFunctionType.Ln)
                nc.scalar.activation(nt[:, 512:1024], nt[:, 0:512],
                                     mybir.ActivationFunctionType.Exp,
                                     scale=-1.0)
                ctile = ctxp.tile([128, 512], BF16, name=f"ctx{s}_{p}", tag="ctx")
                nc.vector.tensor_tensor(
                    ctile[:], pc[:], nt[:, 512:1024], op=mybir.AluOpType.mult)
                out.append(ctile)

            # output projection group g=(t,n) of seq s
            def o_finish(s, g, po):
                t, n = g // NCH, g % NCH
                ob = osbp.tile([128, 512], F32, name=f"ob{s}{t}{n}", tag="ob")
                nc.vector.tensor_tensor(
                    ob[:], po[:, 0:512], bob[:, n * 512:(n + 1) * 512],
                    op=mybir.AluOpType.add)
                nc.sync.dma_start(
                    os_d[s * C + t * 128: s * C + (t + 1) * 128,
                         n * 512:(n + 1) * 512],
                    ob[:])

            def o_chunks(g, po, ctx_tiles, ks, stop):
                t, n = g // NCH, g % NCH
                for k in ks:
                    nc.tensor.matmul(
                        po[:],
                        ctx_tiles[k][:, t * 128:(t + 1) * 128],
                        wo_sb[k // 4][:, (k % 4) * E + n * 512:
                                      (k % 4) * E + (n + 1) * 512],
                        start=(k == 0), stop=(stop and k == ks[-1]),
                        skip_group_check=True)

            def o_group(s, g, ctx_tiles):
                po = ps_a.tile([128, 512], F32, name=f"po{s}{g}", tag="pj")
                o_chunks(g, po, ctx_tiles, list(range(KCH)), True)
                o_finish(s, g, po)

            # ---------------- main pipeline ----------------
            # Each pair's PV/denominator unit is appended to `pending`
            # and emitted inside a LATER pair's slot so the in-order PE
            # stream never waits on the exps it consumes.
            prev_ctx = None            # seq s-1 ctx tiles for O-proj
            pending = []               # [(s, p, PT2, vsb, out), ...]

            def flush_one_pending(pl_pool=None, pl_tag="pj"):
                if pending:
                    s_, p_, PT2_, vsb_, out_ = pending.pop(0)
                    pc, pl = pv_pair_mm(s_, p_, PT2_, vsb_, pl_pool, pl_tag)
                    pv_pair_chain(s_, p_, pc, pl, out_)

            def make_flush_spans():
                # span-level closures for the pending PV/denominator unit
                # so its matmuls can interleave between the S spans (the
                # S row-tile LDWEIGHTS pairs can't both hide behind their
                # own 213ns span; a partner span gives them the window).
                if not pending:
                    return [], None
                s_, p_, PT2_, vsb_, out_ = pending.pop(0)
                pc = ps_c.tile([128, 512], F32, name=f"pc{s_}{p_}", tag="c")
                box = {}

                def pv_span(c):
                    for hh in range(2):
                        h = 2 * p_ + hh
                        nc.tensor.matmul(
                            pc[64 * hh:64 * hh + 64, :],
                            vsb_[c][:, h * D:(h + 1) * D],
                            PT2_[hh][c // 2][:,
                                             (c % 2) * 512:(c % 2 + 1) * 512],
                            start=(c == 0), stop=(c == TCH - 1),
                            tile_position=(0, 64 * hh),
                            skip_group_check=True)

                def pl_span(c):
                    if "pl" not in box:
                        box["pl"] = ps_a.tile([128, 512], F32,
                                              name=f"pl{s_}{p_}", tag="pj")
                    for hh in range(2):
                        nc.tensor.matmul(
                            box["pl"][64 * hh:64 * hh + 64, :],
                            ones_col[:],
                            PT2_[hh][c // 2][:,
                                             (c % 2) * 512:(c % 2 + 1) * 512],
                            start=(c == 0), stop=(c == TCH - 1),
                            tile_position=(0, 64 * hh),
                            skip_group_check=True)

                spans = ([lambda c=c: pv_span(c) for c in range(TCH)]
                         + [lambda c=c: pl_span(c) for c in range(TCH)])
                chain = lambda: pv_pair_chain(s_, p_, pc, box["pl"], out_)
                return spans, chain

            def flush_chains():
                # the Ln/Exp/normalize chain is emitted at the END of the
                # slot so it sits BEHIND the slot's exps in the strict-
                # FIFO ACT queue (the exps gate ps_s recycling; the chain
                # has a whole extra slot of slack).
                while chains:
                    pv_pair_chain(*chains.pop(0))

            for s in range(SEQS):
                ctx_tiles = []
                if s == 0:
                    # seq 0: QK/S first (fp8 weights arrive first), one
                    # QK pair ahead so S(p) covers qt bias-add latency.
                    # All 8 V groups are emitted at p==4 (wv/xt have
                    # streamed in by then); PV flushes start at p==5
                    # (flush-first in the slot: those pendings' chains
                    # are long done and the PT pool needs the slots).
                    vsb = v_tiles(0)
                    qkt = {0: qk_proj(0, 0, x8)}
                    for p in range(PAIRS):
                        if p + 1 < PAIRS:
                            qkt[p + 1] = qk_proj(0, p + 1, x8)
                        if p == 4:
                            for t in range(TCH):
                                v_group(0, vsb[t], t, 0, xT)
                                v_group(0, vsb[t], t, 1, xT)
                        QT, KT = qkt.pop(p)
                        if p >= 4:
                            flush_one_pending()
                        pse0, pso0 = s_block(0, p, 0, QT, KT)
                        if p >= 4:
                            flush_one_pending()
                        pts0 = s_exps(0, p, 0, pse0, pso0)
                        pse1, pso1 = s_block(0, p, 1, QT, KT)
                        pts1 = s_exps(0, p, 1, pse1, pso1)
                        pending.append(
                            (0, p, [[pts0[0], pts1[0]], [pts0[1], pts1[1]]],
                             vsb, ctx_tiles))
                    # drain the remaining pendings
                    flush_one_pending()
                    x8 = x8_dma(1)
                    flush_one_pending()
                    xT = xT_dma(1)
                    wo_sb = load_w("wo")
                    flush_one_pending()
                else:
                    vsb = v_proj(s, xT)
                    # the cross-seq pending flushes here: V(s)'s matmuls
                    # covered the last exps of seq s-1, so its PV never
                    # stalls the PE.
                    flush_one_pending()
                    if s + 1 < SEQS:
                        x8_next = x8_dma(s + 1)
                        xT_next = xT_dma(s + 1)
                    for p in range(PAIRS):
                        QT, KT = qk_proj(s, p, x8)
                        # O groups of seq s-1 between the K projection and
                        # S: covers the KT bias-add DVE latency.
                        if p == 1:
                            o_group(s - 1, 0, prev_ctx)
                            o_group(s - 1, 1, prev_ctx)
                        elif p >= 2:
                            o_group(s - 1, p, prev_ctx)
                        pse0, pso0 = s_block(s, p, 0, QT, KT)
                        flush_one_pending()
                        pts0 = s_exps(s, p, 0, pse0, pso0)
                        pse1, pso1 = s_block(s, p, 1, QT, KT)
                        flush_one_pending()
                        pts1 = s_exps(s, p, 1, pse1, pso1)
                        pending.append(
                            (s, p, [[pts0[0], pts1[0]], [pts0[1], pts1[1]]],
                             vsb, ctx_tiles))
                    if s + 1 < SEQS:
                        x8 = x8_next
                        xT = xT_next
                prev_ctx = ctx_tiles

            # Tail flush: pair 7's PV/denominator unit is still
            # pending and seq 3's O projection remains.  Two O groups run
            # k=0..5 partials (they only need ctx pairs 0..5) around the
            # pending unit to cover its exps/chain; the tail pl comes
            # from ps_s (free once the last exps drain -- ps_a slots are
            # held by the partials until their o_finish).
            s3 = SEQS - 1

            def o_partial(g, pool, tag):
                po = pool.tile([128, 512], F32, name=f"pot{g}", tag=tag)
                o_chunks(g, po, prev_ctx, list(range(KCH - 2)), False)
                return po

            po0 = o_partial(0, ps_a, "pj")
            flush_one_pending(ps_s, "s")
            po1 = o_partial(1, ps_a, "pj")
            tail_pos = ((0, po0), (1, po1))
            for g, po in tail_pos:
                o_chunks(g, po, prev_ctx, [KCH - 2], False)
            for g, po in tail_pos:
                o_chunks(g, po, prev_ctx, [KCH - 1], True)
                o_finish(s3, g, po)
            for g in range(2, TCH * NCH):
                o_group(s3, g, prev_ctx)

    nc.compile()
    return nc


_NC_CACHE = {}


def get_nc():
    if "nc" not in _NC_CACHE:
        _NC_CACHE["nc"] = build_nc()
    return _NC_CACHE["nc"]


def make_in_maps(x, wq, bq, wk, bk, wv, bv, wo, bo):
    bf = ml_dtypes.bfloat16
    f8 = ml_dtypes.float8_e4m3
    x = np.asarray(x, dtype=np.float32)
    args = {}
    # bf16 wv/wo, 4 k-chunks per tile:
    # w[h, kf, (k%4)*1024 + e] = w[k*128+kf, e],  k = 4h..4h+3
    for n, v in (("wv", wv), ("wo", wo)):
        wb = np.asarray(v, dtype=np.float32).astype(bf)
        args[n] = np.ascontiguousarray(
            wb.reshape(2, 4, 128, E).transpose(0, 2, 1, 3)
            .reshape(2, 128, 4 * E))
    for n, v in (("bv", bv), ("bo", bo)):
        args[n] = np.asarray(v, dtype=np.float32)
    # fp8 Q/K weights, scaled by WS, 2 p-slices per tile:
    # w8[g, kf, pp*1024 + k*128 + pc] = (w*WS)[k*128+kf, (2g+pp)*128+pc]
    for n, v in (("wq8", wq), ("wk8", wk)):
        w8 = (np.asarray(v, np.float32) * WS).astype(f8)
        args[n] = np.ascontiguousarray(
            w8.reshape(KCH, 128, PAIRS // 2, 2, 128)
            .transpose(2, 1, 3, 0, 4).reshape(PAIRS // 2, 128, 2 * KCH * 128))
    for n, v in (("bq", bq), ("bk", bk)):
        b16 = np.asarray(v, dtype=np.float32) * WS
        args[n] = b16
        args[n + "t"] = np.ascontiguousarray(b16.reshape(KCH, 128).T)
    xf = x.reshape(B * R, C, E)
    in_maps = []
    for c in range(NCORES):
        m = dict(args)
        xc = xf[c * SEQS:(c + 1) * SEQS]                  # [4, 512, 1024]
        # x^T per seq: t8[s, kf, k*512+t] = x[s, t, k*128+kf]
        xtr = (xc.transpose(0, 2, 1).reshape(SEQS, KCH, 128, C)
               .transpose(0, 2, 1, 3).reshape(SEQS, 128, KCH * C))
        m["xt"] = np.ascontiguousarray(xtr.astype(bf))
        m["x8"] = np.ascontiguousarray(xtr.astype(f8))
        in_maps.append(m)
    return in_maps


def kernel(x, wq, bq, wk, bk, wv, bv, wo, bo):
    in_maps = make_in_maps(x, wq, bq, wk, bk, wv, bv, wo, bo)
    nc = get_nc()
    res = bass_utils.run_bass_kernel_spmd(
        nc, in_maps, core_ids=list(range(NCORES)))
    out = np.concatenate(
        [res.results[c]["os"].reshape(SEQS, C, E) for c in range(NCORES)], axis=0)
    return out.reshape(B, R, C, E).astype(np.float32)
